# revision 38
# baseline (speedup 1.0000x reference)
"""Distributed Bass kernel for chunked ABC attention on 8 TRN2 NeuronCores.

Sharding: head-parallel. 16 heads / 8 cores = 2 heads per core. Each core
receives the full hidden_states plus its column-shard of Wq/Wk/Wv/Wg/Ws and
row-shard of Wo, computes its two heads end to end, and writes a partial
[T, D] output; the host sums the 8 partials (no on-device collectives).

Math (per head, validated against the jax reference):
  w_j = exp(s_j); W_t = cumsum_j<=t w_j  (= exp(cumlogsumexp))
  ok[t,m]  = (1/W_t[m]) * sum_{j<=t} (q_t.k_j*scale) w_j[m]
  pv       = softmax_m(ok); denominator deferred exactly into the rmsnorm:
             rmsnorm(ov/den) = ov * rsqrt(mean(ov^2) + EPS*den^2)
  ov[t,:]  = sum_m eok[t,m] (1/W_t[m]) sum_{j<=t} w_j[m] v_j   (unnormalized)
  out      = rmsnorm(ov)*silu(g) @ Wo'   (gnw folded into Wo rows host-side)
Chunked over T in blocks of C=128 with running-sum states
  Hk[dk,m] += k^T w,  Hv[m,v] += w^T v  (precomputed snapshot prefix pass).

Perf structure:
  - q/k/s projections run in fp8 e4m3 with DoubleRow perf mode (2 k-tiles per
    matmul); scales (hs*4, W*128) are unfolded in the psum post-ops. Verified
    on host: fp8 on q/k/s leaves rel err at ~4.3e-3 (softmax washes it out);
    v/g/o paths must stay bf16.
  - v is projected directly time-major (data-stationary, weights-moving),
    removing the per-chunk PE transposes + copies of the v path.
  - single activation table (natural_log_exp_and_others) for the whole
    kernel: exp for w/eok, silu via g*recip(1+exp(-g)) (DVE approx recip),
    rstd via exp(-0.5*ln(ms)). No ACT_TABLE_LOAD churn.
  - pass 2 (normalize/gate/out-proj) chunks of block g-1 are interleaved at
    stage boundaries of block g's pass 1, filling the PE during the pass-1
    dependency chains; the kernel tail is only the last block's pass 2 with
    its rms stats issued per chunk behind the ovp matmuls.
  - PE warmup matmuls run under the initial DMA wait and across the final
    pass-1/pass-2 boundary so the array never falls back to the mid pstate
    (cold PE runs at 1.2GHz vs 2.4GHz warm; DoubleRow at speed is 215ns for
    a 2x128x512 matmul, cold it is 427ns).
  - all dram params are laid out partition-major on the host so every DMA is
    one contiguous run per partition; block-0 fp8 data and Wq are triggered
    first so the first projection starts as early as possible.
"""

import numpy as np

T, D = 2048, 2048
H, DK, DV, M = 16, 64, 128, 64
C = 128                      # time chunk
NCH = T // C                 # 16 chunks
DT = D // 128                # 16 contraction tiles
NB = T // 512                # free-dim blocks for projections (over T)
DB = D // 512                # output-feature blocks for the out projection
GRP = 4                      # chunks per block
EPS = 1e-5
SCALE = DK ** -0.5
HS_S = 4.0                   # fp8 scale on hidden_states
W_S = 128.0                  # fp8 scale on Wq/Wk/Ws
S8 = HS_S * W_S
N_CORES = 8

_CACHE = {}


def _build():
    import concourse.bass as bass
    import concourse.bacc as bacc
    import concourse.mybir as mybir
    from concourse.tile import TileContext
    from concourse.masks import make_identity, make_upper_triangular
    from contextlib import ExitStack

    f32 = mybir.dt.float32
    bf16 = mybir.dt.bfloat16
    fp8 = mybir.dt.float8e4
    DR = mybir.MatmulPerfMode.DoubleRow
    Exp = mybir.ActivationFunctionType.Exp
    Ln = mybir.ActivationFunctionType.Ln
    Square = mybir.ActivationFunctionType.Square

    # Force every Exp/Ln onto the one table set that has both, so the act
    # table is loaded exactly once. Set ids are positional, so only the
    # function sets are shrunk — never reordered.
    if not getattr(bacc, "_act_tables_pinned", False):
        _orig_get_tables = bacc.get_activation_tables

        def _pinned_tables(arch):
            t = _orig_get_tables(arch)
            for name, fns in t.items():
                if name != "natural_log_exp_and_others":
                    fns.discard(Exp)
                    fns.discard(Ln)
            return t

        bacc.get_activation_tables = _pinned_tables
        bacc._act_tables_pinned = True

    nc = bacc.Bacc()
    hst8_e = nc.declare_dram_parameter("hst8", [128, NB, DT, 512], fp8, isOutput=False)
    hsb_e = nc.declare_dram_parameter("hsb", [128, NB, DT, 512], bf16, isOutput=False)
    wq_e = nc.declare_dram_parameter("wq", [128, DT, 2 * DK], fp8, isOutput=False)
    wk_e = nc.declare_dram_parameter("wk", [128, DT, 2 * DK], fp8, isOutput=False)
    ws_e = nc.declare_dram_parameter("ws", [128, DT, 2 * M], fp8, isOutput=False)
    wv_e = nc.declare_dram_parameter("wv", [128, DT, 2 * DV], bf16, isOutput=False)
    wg_e = nc.declare_dram_parameter("wg", [128, DT, 2 * DV], bf16, isOutput=False)
    wo_e = nc.declare_dram_parameter("wo", [128, 2, D], bf16, isOutput=False)
    out_e = nc.declare_dram_parameter("out", [T, D], bf16, isOutput=True)

    with TileContext(nc) as tc, ExitStack() as ctx:
        consts = ctx.enter_context(tc.tile_pool(name="consts", bufs=1))
        wpool = ctx.enter_context(tc.tile_pool(name="weights", bufs=1))
        big = ctx.enter_context(tc.tile_pool(name="big", bufs=1))
        hsb_pool = ctx.enter_context(tc.tile_pool(name="hsb", bufs=2))
        work = ctx.enter_context(tc.tile_pool(name="work", bufs=4))
        gwork = ctx.enter_context(tc.tile_pool(name="gwork", bufs=2))
        scal = ctx.enter_context(tc.tile_pool(name="scal", bufs=4))
        snapp = ctx.enter_context(tc.tile_pool(name="snap", bufs=2))
        snapstore = ctx.enter_context(tc.tile_pool(name="snapstore", bufs=1))
        outp = ctx.enter_context(tc.tile_pool(name="outp", bufs=4))
        ps_pj = ctx.enter_context(tc.tile_pool(name="ps_pj", bufs=2, space="PSUM"))
        ps_tr = ctx.enter_context(tc.tile_pool(name="ps_tr", bufs=2, space="PSUM"))
        ps_mm = ctx.enter_context(tc.tile_pool(name="ps_mm", bufs=4, space="PSUM"))

        # ---- DMAs: block-0 fp8 data + q/k/s weights first (they gate the
        # first matmuls), then everything else. All transfers are contiguous
        # per partition. ----
        hst8 = big.tile([128, NB, DT, 512], fp8)
        nc.sync.dma_start(out=hst8[:, 0, 0:4], in_=hst8_e[:, 0, 0:4])
        wq_s = wpool.tile([128, DT, 128], fp8, tag="wq")
        nc.sync.dma_start(out=wq_s, in_=wq_e[:, :, :])
        nc.sync.dma_start(out=hst8[:, 0, 4:8], in_=hst8_e[:, 0, 4:8])
        nc.sync.dma_start(out=hst8[:, 0, 8:16], in_=hst8_e[:, 0, 8:16])
        wk_s = wpool.tile([128, DT, 128], fp8, tag="wk")
        nc.sync.dma_start(out=wk_s, in_=wk_e[:, :, :])
        ws_s = wpool.tile([128, DT, 128], fp8, tag="ws")
        nc.sync.dma_start(out=ws_s, in_=ws_e[:, :, :])

        hsb_t = {}
        def hsb_stage(nb):
            t = hsb_pool.tile([128, DT, 512], bf16, tag="hsb")
            nc.sync.dma_start(out=t, in_=hsb_e[:, nb])
            hsb_t[nb] = t
        hsb_stage(0)

        wv_s = wpool.tile([128, DT, 256], bf16, tag="wv")
        nc.sync.dma_start(out=wv_s, in_=wv_e[:, :, :])
        wg_s = wpool.tile([128, DT, 256], bf16, tag="wg")
        nc.sync.dma_start(out=wg_s, in_=wg_e[:, :, :])
        for nb in range(1, NB):
            nc.sync.dma_start(out=hst8[:, nb], in_=hst8_e[:, nb])
        wo_s = wpool.tile([128, 2, D], bf16)
        nc.sync.dma_start(out=wo_s, in_=wo_e[:, :, :])

        # ---- PE warmup: ~4us of junk matmuls while the first DMAs land.
        # The PE needs ~3us of continuous work to leave the low/mid pstate;
        # without this the first two projections run at 1.2GHz instead of
        # 2.4GHz. Results go to a scratch psum tile nobody reads. ----
        warm = consts.tile([128, 512], bf16)
        nc.vector.memset(warm, 0.0)

        def pe_warm(n):
            for _ in range(n):
                wps = ps_pj.tile([128, 512], f32, tag="pj")
                nc.tensor.matmul(wps, lhsT=warm[:, 0:128], rhs=warm,
                                 start=True, stop=True)
        pe_warm(18)

        # ---- constants ----
        ident = consts.tile([128, 128], bf16)
        make_identity(nc, ident)
        maskT = consts.tile([128, 128], bf16)          # maskT[j,t] = 1 if j<=t
        make_upper_triangular(nc, maskT, val=1.0, diag=True)
        zer512 = consts.tile([128, 512], f32)
        nc.vector.memset(zer512, 0.0)

        # ---- persistent activations (feature-major, 2 heads stacked) ----
        qT2 = big.tile([128, T], bf16)                # rows h*64+dk
        kT2 = big.tile([128, T], bf16)
        wT2 = big.tile([128, T], bf16)                # exp(s), rows h*64+m
        WinvT2 = big.tile([128, T], f32)              # 1 / cumsum(exp(s))
        v_tm = big.tile([128, NCH, 256], bf16)        # [t', chunk, h*128+v]
        sw = big.tile([128, 2, T], bf16)              # silu(g), rows: v
        w_tm = big.tile([128, NCH, 128], bf16)        # [j, chunk, h*64+m]
        k_tm = big.tile([128, NCH, 128], bf16)        # [j, chunk, h*64+dk]
        ov_all = big.tile([128, NCH, 256], bf16)      # [t, chunk, h*128+v]
        ms_all = big.tile([128, NCH, 2], f32)         # mean(ov^2) + EPS*den^2
        rstd_all = big.tile([128, NCH, 2], f32)

        snaps = []
        state = {"snapf_prev": None, "Wprev": None}

        def proj8(w8, nb):
            # fp8 DoubleRow projection: 8 matmuls cover the 2048 contraction
            ps = ps_pj.tile([128, 512], f32, tag="pj")
            for dd in range(DT // 2):
                nc.tensor.matmul(
                    ps,
                    lhsT=w8[:, 2 * dd:2 * dd + 2, :],
                    rhs=hst8[:, nb, 2 * dd:2 * dd + 2, :],
                    start=(dd == 0), stop=(dd == DT // 2 - 1),
                    perf_mode=DR,
                )
            return ps

        def projb(w_s, h, nb):
            # bf16 feature-major projection (gate)
            ps = ps_pj.tile([128, 512], f32, tag="pj")
            for dd in range(DT):
                nc.tensor.matmul(
                    ps,
                    lhsT=w_s[:, dd, h * 128:h * 128 + 128],
                    rhs=hsb_t[nb][:, dd, :],
                    start=(dd == 0), stop=(dd == DT - 1),
                )
            return ps

        def pass2_chunk(c):
            og = work.tile([128, 2, 128], bf16, tag="og")
            for h in range(2):
                # normalize on scalar: rstd was just computed there, so the
                # chain stays on one queue with no cross-engine semaphore hop
                o_n = work.tile([128, 128], bf16, tag="on")
                nc.scalar.mul(out=o_n, in_=ov_all[:, c, h * 128:(h + 1) * 128],
                              mul=rstd_all[:, c, h:h + 1])
                pst = ps_tr.tile([128, 128], bf16, tag="tr")
                nc.tensor.transpose(pst, o_n, ident)
                nc.vector.tensor_mul(og[:, h, :], pst,
                                     sw[:, h, c * 128:(c + 1) * 128])
            orow = outp.tile([128, D], bf16, tag="orow")
            for nbp in range(DB // 2):
                nb0, nb1 = 2 * nbp, 2 * nbp + 1
                ps_a = ps_pj.tile([128, 512], f32, tag="pj")
                ps_b = ps_pj.tile([128, 512], f32, tag="pj")
                # og[h] stays loaded in the PE across both output blocks
                nc.tensor.matmul(ps_a, lhsT=og[:, 0, :],
                                 rhs=wo_s[:, 0, nb0 * 512:(nb0 + 1) * 512],
                                 start=True, stop=False)
                nc.tensor.matmul(ps_b, lhsT=og[:, 0, :],
                                 rhs=wo_s[:, 0, nb1 * 512:(nb1 + 1) * 512],
                                 start=True, stop=False)
                nc.tensor.matmul(ps_a, lhsT=og[:, 1, :],
                                 rhs=wo_s[:, 1, nb0 * 512:(nb0 + 1) * 512],
                                 start=False, stop=True)
                nc.tensor.matmul(ps_b, lhsT=og[:, 1, :],
                                 rhs=wo_s[:, 1, nb1 * 512:(nb1 + 1) * 512],
                                 start=False, stop=True)
                nc.scalar.copy(out=orow[:, nb0 * 512:(nb0 + 1) * 512],
                               in_=ps_a)
                nc.vector.tensor_copy(
                    out=orow[:, nb1 * 512:(nb1 + 1) * 512], in_=ps_b)
            nc.sync.dma_start(out=out_e[c * 128:(c + 1) * 128, :], in_=orow)

        # ---- main block loop ----
        # NOTE (HW quirk, repro'd): a matmul whose PSUM out has 128 partitions
        # crashes the exec unit when the out column offset is nonzero; M=64
        # col-offset outs are fine. All M=128 matmul outs below sit at the
        # base of their own pool tile.
        for g in range(NB):
            nb = g
            blk = slice(nb * 512, (nb + 1) * 512)

            # fp8 q/k/s projections; unscale folded into psum post-ops
            ps = proj8(wq_s, nb)
            nc.scalar.mul(out=qT2[:, blk], in_=ps, mul=SCALE / S8)
            ps = proj8(wk_s, nb)
            nc.scalar.mul(out=kT2[:, blk], in_=ps, mul=1.0 / S8)
            ps = proj8(ws_s, nb)
            nc.scalar.activation(out=wT2[:, blk], in_=ps, func=Exp,
                                 scale=1.0 / S8)
            # pass 2 of the previous block is interleaved into this block's
            # stages (chunk at a time) so its tensor work fills dependency
            # stalls in the pass-1 chains.
            prev = [g * GRP - GRP + i for i in range(GRP)] if g >= 1 else []
            if prev:
                pass2_chunk(prev[0])
            else:
                pe_warm(3)
            if g + 1 < NB:
                hsb_stage(g + 1)

            # running normalizer W = cumsum(w) along t, chained across blocks.
            # Issued after the pass-2 interleave: Winv isn't needed until the
            # okp stage, and the scan's ~1.2us on vector would otherwise
            # delay pass-2's normalize/gate chain.
            Wb = work.tile([128, 512], f32, tag="Wb")
            nc.vector.tensor_tensor_scan(
                out=Wb, data0=wT2[:, blk], data1=zer512,
                initial=(0.0 if nb == 0 else state["Wprev"][:, 511:512]),
                op0=mybir.AluOpType.add, op1=mybir.AluOpType.add)
            nc.vector.reciprocal_approx_fast(out=WinvT2[:, blk], in_=Wb)
            state["Wprev"] = Wb

            if not prev:
                pe_warm(6)

            # v: direct time-major projection (data stationary, wv moving)
            for i in range(GRP):
                tcb = g * GRP + i
                psv = ps_mm.tile([128, 256], f32, tag="mm")
                for dd in range(DT):
                    nc.tensor.matmul(
                        psv,
                        lhsT=hsb_t[nb][:, dd, i * 128:(i + 1) * 128],
                        rhs=wv_s[:, dd, :],
                        start=(dd == 0), stop=(dd == DT - 1),
                    )
                nc.scalar.copy(out=v_tm[:, tcb, :], in_=psv)

            if prev:
                pass2_chunk(prev[1])
            else:
                pe_warm(3)

            def gate_block():
                # gate: sw = g*sigmoid(g) via exp (same act table), recip on
                # DVE. For blocks 0..NB-2 this is issued mid pass-1 where its
                # tensor work bridges the okp dependency stall; for the last
                # block it runs early so its scalar ops don't delay the rms
                # stats that gate the kernel tail.
                for h in range(2):
                    psg = projb(wg_s, h, nb)
                    e_b = gwork.tile([128, 512], bf16, tag="e")
                    nc.scalar.activation(out=e_b, in_=psg, func=Exp, scale=-1.0)
                    g_b = gwork.tile([128, 512], bf16, tag="g")
                    nc.scalar.copy(out=g_b, in_=psg)
                    t1 = gwork.tile([128, 512], f32, tag="t1")
                    nc.vector.tensor_scalar(out=t1, in0=e_b, scalar1=1.0,
                                            scalar2=None, op0=mybir.AluOpType.add)
                    t2 = gwork.tile([128, 512], f32, tag="t2")
                    nc.vector.reciprocal_approx_fast(out=t2, in_=t1)
                    nc.vector.tensor_mul(sw[:, h, blk], g_b, t2)

            if g == NB - 1:
                gate_block()

            # time-major transposes of w and k for this block's chunks
            for tcb in range(g * GRP, (g + 1) * GRP):
                cblk = slice(tcb * 128, (tcb + 1) * 128)
                pst = ps_tr.tile([128, 128], bf16, tag="tr")
                nc.tensor.transpose(pst, wT2[:, cblk], ident)
                nc.vector.tensor_copy(out=w_tm[:, tcb, :], in_=pst)
                pst = ps_tr.tile([128, 128], bf16, tag="tr")
                nc.tensor.transpose(pst, kT2[:, cblk], ident)
                nc.scalar.copy(out=k_tm[:, tcb, :], in_=pst)

            # snapshot prefix: snaps[c] = state after chunks 0..c
            for tcb in range(g * GRP, min((g + 1) * GRP, NCH - 1)):
                u_ps = ps_mm.tile([128, 256], f32, tag="mm")
                for h in range(2):
                    hp = slice(h * 64, (h + 1) * 64)
                    nc.tensor.matmul(u_ps[hp, 0:64], lhsT=k_tm[:, tcb, hp],
                                     rhs=w_tm[:, tcb, hp], start=True, stop=True)
                for h in range(2):
                    hp = slice(h * 64, (h + 1) * 64)
                    nc.tensor.matmul(u_ps[hp, 64:192],
                                     lhsT=w_tm[:, tcb, hp],
                                     rhs=v_tm[:, tcb, h * 128:(h + 1) * 128],
                                     start=True, stop=True)
                snapf = snapp.tile([128, 192], f32, tag="snapf")
                if tcb == 0:
                    nc.vector.tensor_copy(out=snapf, in_=u_ps[:, 0:192])
                else:
                    nc.vector.tensor_add(snapf, state["snapf_prev"],
                                         u_ps[:, 0:192])
                snapb = snapstore.tile([128, 192], bf16, tag=f"s{tcb}")
                nc.gpsimd.tensor_copy(out=snapb, in_=snapf)
                state["snapf_prev"] = snapf
                snaps.append(snapb)

            # ---- pass 1 for this block's chunks (stage-major) ----
            chunks = range(g * GRP, (g + 1) * GRP)
            blks = {c: slice(c * 128, (c + 1) * 128) for c in chunks}

            # slot logits per head: atm[j, t] = mask * k^T q
            aps_t, atm_t = {}, {}
            for c in chunks:
                for h in range(2):
                    hp = slice(h * 64, (h + 1) * 64)
                    aps = ps_mm.tile([128, 128], f32, tag="mm")
                    nc.tensor.matmul(aps, lhsT=kT2[hp, blks[c]],
                                     rhs=qT2[hp, blks[c]], start=True, stop=True)
                    aps_t[c, h] = aps
            for c in chunks:
                atm = work.tile([128, 256], bf16, tag="atm")
                for h in range(2):
                    nc.vector.tensor_mul(atm[:, h * 128:(h + 1) * 128],
                                         aps_t[c, h], maskT)
                atm_t[c] = atm

            if g < NB - 1:
                gate_block()

            if prev:
                pass2_chunk(prev[2])

            okp_t = {}
            for c in chunks:
                okp = ps_mm.tile([128, 128], f32, tag="mm")
                # issue the two heads' matmuls adjacently per stage so the
                # col-tiled halves can run concurrently in the PE array
                for h in range(2):
                    hp = slice(h * 64, (h + 1) * 64)
                    nc.tensor.matmul(okp[hp, :], lhsT=w_tm[:, c, hp],
                                     rhs=atm_t[c][:, h * 128:(h + 1) * 128],
                                     start=True, stop=c == 0)
                if c > 0:
                    for h in range(2):
                        hp = slice(h * 64, (h + 1) * 64)
                        nc.tensor.matmul(okp[hp, :], lhsT=snaps[c - 1][hp, 0:64],
                                         rhs=qT2[hp, blks[c]],
                                         start=False, stop=True)
                okp_t[c] = okp
            eok_t = {}
            for c in chunks:
                ok_n = work.tile([128, 128], f32, tag="okn")
                nc.vector.tensor_mul(ok_n, okp_t[c], WinvT2[:, blks[c]])
                eok_t[c] = ok_n
            for c in chunks:
                eok = work.tile([128, 128], bf16, tag="eok")
                nc.scalar.activation(out=eok, in_=eok_t[c], func=Exp)
                eok_t[c] = eok

            # deferred softmax denominator: dsq = EPS * den^2 per head
            pde_t, dsq_t, pvw_t = {}, {}, {}
            for c in chunks:
                pde = ps_tr.tile([128, 128], bf16, tag="tr")
                nc.tensor.transpose(pde, eok_t[c], ident)
                pde_t[c] = pde
            for c in chunks:
                dn = scal.tile([128, 2], f32, tag="dn")
                for h in range(2):
                    nc.vector.tensor_reduce(out=dn[:, h:h + 1],
                                            in_=pde_t[c][:, h * 64:(h + 1) * 64],
                                            axis=mybir.AxisListType.X,
                                            op=mybir.AluOpType.add)
                dsq = scal.tile([128, 2], f32, tag="dsq")
                nc.vector.tensor_scalar(out=dsq, in0=dn, scalar1=EPS,
                                        scalar2=None, op0=mybir.AluOpType.mult)
                nc.vector.tensor_mul(dsq, dsq, dn)
                dsq_t[c] = dsq
                pvw = work.tile([128, 128], bf16, tag="pvw")
                nc.vector.tensor_mul(pvw, eok_t[c], WinvT2[:, blks[c]])
                pvw_t[c] = pvw

            if prev:
                pass2_chunk(prev[3])

            pps_t, ptm_t = {}, {}
            for c in chunks:
                for h in range(2):
                    hp = slice(h * 64, (h + 1) * 64)
                    pps = ps_mm.tile([128, 128], f32, tag="mm")
                    nc.tensor.matmul(pps, lhsT=wT2[hp, blks[c]],
                                     rhs=pvw_t[c][hp, :], start=True, stop=True)
                    pps_t[c, h] = pps
            for c in chunks:
                ptm = work.tile([128, 256], bf16, tag="ptm")
                for h in range(2):
                    nc.vector.tensor_mul(ptm[:, h * 128:(h + 1) * 128],
                                         pps_t[c, h], maskT)
                ptm_t[c] = ptm

            # ovp matmuls with the rms-stat chain issued per chunk right
            # behind them, so the scalar queue drains while later chunks'
            # matmuls still run (shortens the kernel tail).
            for c in chunks:
                ovp_t = {}
                for h in range(2):
                    hp = slice(h * 64, (h + 1) * 64)
                    ovp = ps_mm.tile([128, 128], f32, tag="mm")
                    nc.tensor.matmul(ovp,
                                     lhsT=ptm_t[c][:, h * 128:(h + 1) * 128],
                                     rhs=v_tm[:, c, h * 128:(h + 1) * 128],
                                     start=True, stop=c == 0)
                    if c > 0:
                        nc.tensor.matmul(ovp, lhsT=pvw_t[c][hp, :],
                                         rhs=snaps[c - 1][hp, 64:192],
                                         start=False, stop=True)
                    ovp_t[h] = ovp
                msq = scal.tile([128, 2], f32, tag="msq")
                for h in range(2):
                    hb = slice(h * 128, (h + 1) * 128)
                    nc.vector.tensor_copy(out=ov_all[:, c, hb], in_=ovp_t[h])
                    # rms stats: ms = sum(ov^2)/DV + EPS*den^2
                    scr = work.tile([128, 128], bf16, tag="scr")
                    nc.scalar.activation(out=scr, in_=ovp_t[h], func=Square,
                                         accum_out=msq[:, h:h + 1])
                    nc.vector.tensor_scalar(
                        out=ms_all[:, c, h:h + 1], in0=msq[:, h:h + 1],
                        scalar1=1.0 / DV, scalar2=dsq_t[c][:, h:h + 1],
                        op0=mybir.AluOpType.mult, op1=mybir.AluOpType.add)
                # rstd = exp(-0.5 * ln(ms)) -- same act table as Exp; issued
                # here so the last chunks' rstd is ready when pass 2 starts
                lnb = scal.tile([128, 2], f32, tag="lnb")
                nc.scalar.activation(out=lnb, in_=ms_all[:, c, :], func=Ln)
                nc.scalar.activation(out=rstd_all[:, c, :], in_=lnb, func=Exp,
                                     scale=-0.5)

        # ---- tail: pass 2 of the final block. A few junk matmuls bridge the
        # dependency wait on the last chunks' rms stats so the PE stays in
        # the fast pstate for the final out-projections. ----
        for i, c in enumerate(range(NCH - GRP, NCH)):
            pe_warm(12 if i == 0 else 0)
            pass2_chunk(c)

    nc.compile()
    return nc


def _get_nc():
    if "nc" not in _CACHE:
        _CACHE["nc"] = _build()
    return _CACHE["nc"]


def _part_major(a, inner):
    # [D, cols] -> [128, DT, cols] with partition-major contiguous runs
    return np.ascontiguousarray(
        a.reshape(DT, 128, inner).transpose(1, 0, 2))


def _make_in_maps(inputs):
    import ml_dtypes

    bfdt = ml_dtypes.bfloat16
    f8dt = ml_dtypes.float8_e4m3

    hs = np.asarray(inputs["hidden_states"], dtype=np.float32).reshape(T, D)
    # [T, D] -> [128, NB, DT, 512]: element (p, nb, dd, t') = hs[nb*512+t', dd*128+p]
    hs_pm = hs.reshape(NB, 512, DT, 128).transpose(3, 0, 2, 1)
    hst8 = np.ascontiguousarray((hs_pm * HS_S)).astype(f8dt)
    hsb = np.ascontiguousarray(hs_pm).astype(bfdt)

    Wq = np.asarray(inputs["Wq"], dtype=np.float32)
    Wk = np.asarray(inputs["Wk"], dtype=np.float32)
    Wv = np.asarray(inputs["Wv"], dtype=np.float32)
    Wg = np.asarray(inputs["Wg"], dtype=np.float32)
    Ws = np.asarray(inputs["Ws"], dtype=np.float32)
    Wo = np.asarray(inputs["Wo"], dtype=np.float32)
    gnw = np.asarray(inputs["g_norm_weight"], dtype=np.float32)
    # fold gnw into Wo rows: (o_n*gnw*sg) @ Wo == (o_n*sg) @ (gnw[:,None]*Wo)
    Wo_f = Wo * np.tile(gnw, H)[:, None]

    in_maps = []
    for i in range(N_CORES):
        in_maps.append({
            "hst8": hst8,
            "hsb": hsb,
            "wq": _part_major(Wq[:, i * 128:(i + 1) * 128] * W_S, 128).astype(f8dt),
            "wk": _part_major(Wk[:, i * 128:(i + 1) * 128] * W_S, 128).astype(f8dt),
            "ws": _part_major(Ws[:, i * 128:(i + 1) * 128] * W_S, 128).astype(f8dt),
            "wv": _part_major(Wv[:, i * 256:(i + 1) * 256], 256).astype(bfdt),
            "wg": _part_major(Wg[:, i * 256:(i + 1) * 256], 256).astype(bfdt),
            "wo": np.ascontiguousarray(
                Wo_f[i * 256:(i + 1) * 256, :].reshape(2, 128, D)
                .transpose(1, 0, 2)).astype(bfdt),
        })
    return in_maps


def _gather(res):
    out = np.zeros((T, D), np.float32)
    for r in res.results:
        out += np.asarray(r["out"]).astype(np.float32)
    return out.reshape(1, T, D)


def kernel(**inputs):
    from concourse.bass_utils import run_bass_kernel_spmd

    nc = _get_nc()
    in_maps = _make_in_maps(inputs)
    res = run_bass_kernel_spmd(nc, in_maps, core_ids=list(range(N_CORES)))
    return _gather(res)


# revision 39
# speedup vs baseline: 1.0260x; 1.0260x over previous
"""Distributed Bass kernel for chunked ABC attention on 8 TRN2 NeuronCores.

Sharding: head-parallel. 16 heads / 8 cores = 2 heads per core. Each core
receives the full hidden_states plus its column-shard of Wq/Wk/Wv/Wg/Ws and
row-shard of Wo, computes its two heads end to end, and writes a partial
[T, D] output; the host sums the 8 partials (no on-device collectives).

Math (per head, validated against the jax reference):
  w_j = exp(s_j); W_t = cumsum_j<=t w_j  (= exp(cumlogsumexp))
  ok[t,m]  = (1/W_t[m]) * sum_{j<=t} (q_t.k_j*scale) w_j[m]
  pv       = softmax_m(ok); denominator deferred exactly into the rmsnorm:
             rmsnorm(ov/den) = ov * rsqrt(mean(ov^2) + EPS*den^2)
  ov[t,:]  = sum_m eok[t,m] (1/W_t[m]) sum_{j<=t} w_j[m] v_j   (unnormalized)
  out      = rmsnorm(ov)*silu(g) @ Wo'   (gnw folded into Wo rows host-side)
Chunked over T in blocks of C=128 with running-sum states
  Hk[dk,m] += k^T w,  Hv[m,v] += w^T v  (precomputed snapshot prefix pass).

Perf structure:
  - q/k/s projections run in fp8 e4m3 with DoubleRow perf mode (2 k-tiles per
    matmul); scales (hs*4, W*128) are unfolded in the psum post-ops. Verified
    on host: fp8 on q/k/s leaves rel err at ~4.3e-3 (softmax washes it out);
    v/g/o paths must stay bf16.
  - v is projected directly time-major (data-stationary, weights-moving),
    removing the per-chunk PE transposes + copies of the v path.
  - single activation table (natural_log_exp_and_others) for the whole
    kernel: exp for w/eok, silu via g*recip(1+exp(-g)) (DVE approx recip),
    rstd via exp(-0.5*ln(ms)). No ACT_TABLE_LOAD churn.
  - pass 2 (normalize/gate/out-proj) chunks of block g-1 are interleaved at
    stage boundaries of block g's pass 1, filling the PE during the pass-1
    dependency chains; the kernel tail is only the last block's pass 2 with
    its rms stats issued per chunk behind the ovp matmuls.
  - PE warmup matmuls run under the initial DMA wait and across the final
    pass-1/pass-2 boundary so the array never falls back to the mid pstate
    (cold PE runs at 1.2GHz vs 2.4GHz warm; DoubleRow at speed is 215ns for
    a 2x128x512 matmul, cold it is 427ns).
  - all dram params are laid out partition-major on the host so every DMA is
    one contiguous run per partition; block-0 fp8 data and Wq are triggered
    first so the first projection starts as early as possible.
"""

import numpy as np

T, D = 2048, 2048
H, DK, DV, M = 16, 64, 128, 64
C = 128                      # time chunk
NCH = T // C                 # 16 chunks
DT = D // 128                # 16 contraction tiles
NB = T // 512                # free-dim blocks for projections (over T)
DB = D // 512                # output-feature blocks for the out projection
GRP = 4                      # chunks per block
EPS = 1e-5
SCALE = DK ** -0.5
HS_S = 4.0                   # fp8 scale on hidden_states
W_S = 128.0                  # fp8 scale on Wq/Wk/Ws
S8 = HS_S * W_S
N_CORES = 8

_CACHE = {}


def _build():
    import concourse.bass as bass
    import concourse.bacc as bacc
    import concourse.mybir as mybir
    from concourse.tile import TileContext
    from concourse.masks import make_identity, make_upper_triangular
    from contextlib import ExitStack

    f32 = mybir.dt.float32
    bf16 = mybir.dt.bfloat16
    fp8 = mybir.dt.float8e4
    DR = mybir.MatmulPerfMode.DoubleRow
    Exp = mybir.ActivationFunctionType.Exp
    Ln = mybir.ActivationFunctionType.Ln
    Square = mybir.ActivationFunctionType.Square

    # Force every Exp/Ln onto the one table set that has both, so the act
    # table is loaded exactly once. Set ids are positional, so only the
    # function sets are shrunk — never reordered.
    if not getattr(bacc, "_act_tables_pinned", False):
        _orig_get_tables = bacc.get_activation_tables

        def _pinned_tables(arch):
            t = _orig_get_tables(arch)
            for name, fns in t.items():
                if name != "natural_log_exp_and_others":
                    fns.discard(Exp)
                    fns.discard(Ln)
            return t

        bacc.get_activation_tables = _pinned_tables
        bacc._act_tables_pinned = True

    nc = bacc.Bacc()
    hst8_e = nc.declare_dram_parameter("hst8", [128, NB, DT, 512], fp8, isOutput=False)
    hsb_e = nc.declare_dram_parameter("hsb", [128, NB, DT, 512], bf16, isOutput=False)
    wq_e = nc.declare_dram_parameter("wq", [128, DT, 2 * DK], fp8, isOutput=False)
    wk_e = nc.declare_dram_parameter("wk", [128, DT, 2 * DK], fp8, isOutput=False)
    ws_e = nc.declare_dram_parameter("ws", [128, DT, 2 * M], fp8, isOutput=False)
    wv_e = nc.declare_dram_parameter("wv", [128, DT, 2 * DV], bf16, isOutput=False)
    wg_e = nc.declare_dram_parameter("wg", [128, DT, 2 * DV], bf16, isOutput=False)
    wo_e = nc.declare_dram_parameter("wo", [128, 2, D], bf16, isOutput=False)
    out_e = nc.declare_dram_parameter("out", [T, D], bf16, isOutput=True)

    with TileContext(nc) as tc, ExitStack() as ctx:
        consts = ctx.enter_context(tc.tile_pool(name="consts", bufs=1))
        wpool = ctx.enter_context(tc.tile_pool(name="weights", bufs=1))
        big = ctx.enter_context(tc.tile_pool(name="big", bufs=1))
        hsb_pool = ctx.enter_context(tc.tile_pool(name="hsb", bufs=2))
        work = ctx.enter_context(tc.tile_pool(name="work", bufs=4))
        gwork = ctx.enter_context(tc.tile_pool(name="gwork", bufs=2))
        scal = ctx.enter_context(tc.tile_pool(name="scal", bufs=4))
        snapp = ctx.enter_context(tc.tile_pool(name="snap", bufs=2))
        snapstore = ctx.enter_context(tc.tile_pool(name="snapstore", bufs=1))
        outp = ctx.enter_context(tc.tile_pool(name="outp", bufs=4))
        ps_pj = ctx.enter_context(tc.tile_pool(name="ps_pj", bufs=2, space="PSUM"))
        ps_tr = ctx.enter_context(tc.tile_pool(name="ps_tr", bufs=2, space="PSUM"))
        ps_mm = ctx.enter_context(tc.tile_pool(name="ps_mm", bufs=4, space="PSUM"))

        # ---- DMAs: block-0 fp8 data + q/k/s weights first (they gate the
        # first matmuls), then everything else. All transfers are contiguous
        # per partition. ----
        hst8 = big.tile([128, NB, DT, 512], fp8)
        nc.sync.dma_start(out=hst8[:, 0, 0:4], in_=hst8_e[:, 0, 0:4])
        wq_s = wpool.tile([128, DT, 128], fp8, tag="wq")
        nc.sync.dma_start(out=wq_s, in_=wq_e[:, :, :])
        nc.sync.dma_start(out=hst8[:, 0, 4:8], in_=hst8_e[:, 0, 4:8])
        nc.sync.dma_start(out=hst8[:, 0, 8:16], in_=hst8_e[:, 0, 8:16])
        wk_s = wpool.tile([128, DT, 128], fp8, tag="wk")
        nc.sync.dma_start(out=wk_s, in_=wk_e[:, :, :])
        ws_s = wpool.tile([128, DT, 128], fp8, tag="ws")
        nc.sync.dma_start(out=ws_s, in_=ws_e[:, :, :])

        hsb_t = {}
        def hsb_stage(nb):
            t = hsb_pool.tile([128, DT, 512], bf16, tag="hsb")
            nc.sync.dma_start(out=t, in_=hsb_e[:, nb])
            hsb_t[nb] = t
        hsb_stage(0)

        wv_s = wpool.tile([128, DT, 256], bf16, tag="wv")
        nc.sync.dma_start(out=wv_s, in_=wv_e[:, :, :])
        wg_s = wpool.tile([128, DT, 256], bf16, tag="wg")
        nc.sync.dma_start(out=wg_s, in_=wg_e[:, :, :])
        for nb in range(1, NB):
            nc.sync.dma_start(out=hst8[:, nb], in_=hst8_e[:, nb])
        wo_s = wpool.tile([128, 2, D], bf16)
        nc.sync.dma_start(out=wo_s, in_=wo_e[:, :, :])

        # ---- PE warmup: ~4us of junk matmuls while the first DMAs land.
        # The PE needs ~3us of continuous work to leave the low/mid pstate;
        # without this the first two projections run at 1.2GHz instead of
        # 2.4GHz. Results go to a scratch psum tile nobody reads. ----
        warm = consts.tile([128, 512], bf16)
        nc.vector.memset(warm, 0.0)

        def pe_warm(n):
            for _ in range(n):
                wps = ps_pj.tile([128, 512], f32, tag="pj")
                nc.tensor.matmul(wps, lhsT=warm[:, 0:128], rhs=warm,
                                 start=True, stop=True)
        pe_warm(18)

        # ---- constants ----
        ident = consts.tile([128, 128], bf16)
        make_identity(nc, ident)
        maskT = consts.tile([128, 128], bf16)          # maskT[j,t] = 1 if j<=t
        make_upper_triangular(nc, maskT, val=1.0, diag=True)
        zer512 = consts.tile([128, 512], f32)
        nc.vector.memset(zer512, 0.0)

        # ---- persistent activations (feature-major, 2 heads stacked) ----
        qT2 = big.tile([128, T], bf16)                # rows h*64+dk
        kT2 = big.tile([128, T], bf16)
        wT2 = big.tile([128, T], bf16)                # exp(s), rows h*64+m
        WinvT2 = big.tile([128, T], f32)              # 1 / cumsum(exp(s))
        v_tm = big.tile([128, NCH, 256], bf16)        # [t', chunk, h*128+v]
        sw = big.tile([128, 2, T], bf16)              # silu(g), rows: v
        w_tm = big.tile([128, NCH, 128], bf16)        # [j, chunk, h*64+m]
        k_tm = big.tile([128, NCH, 128], bf16)        # [j, chunk, h*64+dk]
        ov_all = big.tile([128, NCH, 256], bf16)      # [t, chunk, h*128+v]
        ms_all = big.tile([128, NCH, 2], f32)         # mean(ov^2) + EPS*den^2
        rstd_all = big.tile([128, NCH, 2], f32)

        snaps = []
        state = {"snapf_prev": None, "Wprev": None}

        def proj8(w8, nb):
            # fp8 DoubleRow projection: 8 matmuls cover the 2048 contraction
            ps = ps_pj.tile([128, 512], f32, tag="pj")
            for dd in range(DT // 2):
                nc.tensor.matmul(
                    ps,
                    lhsT=w8[:, 2 * dd:2 * dd + 2, :],
                    rhs=hst8[:, nb, 2 * dd:2 * dd + 2, :],
                    start=(dd == 0), stop=(dd == DT // 2 - 1),
                    perf_mode=DR,
                )
            return ps

        def projb(w_s, h, nb):
            # bf16 feature-major projection (gate)
            ps = ps_pj.tile([128, 512], f32, tag="pj")
            for dd in range(DT):
                nc.tensor.matmul(
                    ps,
                    lhsT=w_s[:, dd, h * 128:h * 128 + 128],
                    rhs=hsb_t[nb][:, dd, :],
                    start=(dd == 0), stop=(dd == DT - 1),
                )
            return ps

        def pass2_chunk(c):
            og = work.tile([128, 2, 128], bf16, tag="og")
            for h in range(2):
                o_n = work.tile([128, 128], bf16, tag="on")
                nc.vector.tensor_scalar_mul(
                    o_n, ov_all[:, c, h * 128:(h + 1) * 128],
                    rstd_all[:, c, h:h + 1])
                pst = ps_tr.tile([128, 128], bf16, tag="tr")
                nc.tensor.transpose(pst, o_n, ident)
                nc.vector.tensor_mul(og[:, h, :], pst,
                                     sw[:, h, c * 128:(c + 1) * 128])
            orow = outp.tile([128, D], bf16, tag="orow")
            for nbp in range(DB // 2):
                nb0, nb1 = 2 * nbp, 2 * nbp + 1
                ps_a = ps_pj.tile([128, 512], f32, tag="pj")
                ps_b = ps_pj.tile([128, 512], f32, tag="pj")
                # og[h] stays loaded in the PE across both output blocks
                nc.tensor.matmul(ps_a, lhsT=og[:, 0, :],
                                 rhs=wo_s[:, 0, nb0 * 512:(nb0 + 1) * 512],
                                 start=True, stop=False)
                nc.tensor.matmul(ps_b, lhsT=og[:, 0, :],
                                 rhs=wo_s[:, 0, nb1 * 512:(nb1 + 1) * 512],
                                 start=True, stop=False)
                nc.tensor.matmul(ps_a, lhsT=og[:, 1, :],
                                 rhs=wo_s[:, 1, nb0 * 512:(nb0 + 1) * 512],
                                 start=False, stop=True)
                nc.tensor.matmul(ps_b, lhsT=og[:, 1, :],
                                 rhs=wo_s[:, 1, nb1 * 512:(nb1 + 1) * 512],
                                 start=False, stop=True)
                nc.scalar.copy(out=orow[:, nb0 * 512:(nb0 + 1) * 512],
                               in_=ps_a)
                nc.vector.tensor_copy(
                    out=orow[:, nb1 * 512:(nb1 + 1) * 512], in_=ps_b)
            nc.sync.dma_start(out=out_e[c * 128:(c + 1) * 128, :], in_=orow)

        # ---- main block loop ----
        # NOTE (HW quirk, repro'd): a matmul whose PSUM out has 128 partitions
        # crashes the exec unit when the out column offset is nonzero; M=64
        # col-offset outs are fine. All M=128 matmul outs below sit at the
        # base of their own pool tile.
        for g in range(NB):
            nb = g
            blk = slice(nb * 512, (nb + 1) * 512)

            # fp8 q/k/s projections; unscale folded into psum post-ops
            ps = proj8(wq_s, nb)
            nc.scalar.mul(out=qT2[:, blk], in_=ps, mul=SCALE / S8)
            ps = proj8(wk_s, nb)
            nc.scalar.mul(out=kT2[:, blk], in_=ps, mul=1.0 / S8)
            ps = proj8(ws_s, nb)
            nc.scalar.activation(out=wT2[:, blk], in_=ps, func=Exp,
                                 scale=1.0 / S8)
            # pass 2 of the previous block is interleaved into this block's
            # stages (chunk at a time) so its tensor work fills dependency
            # stalls in the pass-1 chains.
            prev = [g * GRP - GRP + i for i in range(GRP)] if g >= 1 else []
            if prev:
                pass2_chunk(prev[0])
            else:
                pe_warm(3)
            if g + 1 < NB:
                hsb_stage(g + 1)

            # running normalizer W = cumsum(w) along t, chained across blocks.
            # Issued after the pass-2 interleave: Winv isn't needed until the
            # okp stage, and the scan's ~1.2us on vector would otherwise
            # delay pass-2's normalize/gate chain.
            Wb = work.tile([128, 512], f32, tag="Wb")
            nc.vector.tensor_tensor_scan(
                out=Wb, data0=wT2[:, blk], data1=zer512,
                initial=(0.0 if nb == 0 else state["Wprev"][:, 511:512]),
                op0=mybir.AluOpType.add, op1=mybir.AluOpType.add)
            nc.vector.reciprocal_approx_fast(out=WinvT2[:, blk], in_=Wb)
            state["Wprev"] = Wb

            if not prev:
                pe_warm(6)

            # v: direct time-major projection (data stationary, wv moving)
            for i in range(GRP):
                tcb = g * GRP + i
                psv = ps_mm.tile([128, 256], f32, tag="mm")
                for dd in range(DT):
                    nc.tensor.matmul(
                        psv,
                        lhsT=hsb_t[nb][:, dd, i * 128:(i + 1) * 128],
                        rhs=wv_s[:, dd, :],
                        start=(dd == 0), stop=(dd == DT - 1),
                    )
                nc.scalar.copy(out=v_tm[:, tcb, :], in_=psv)

            if prev:
                pass2_chunk(prev[1])
            else:
                pe_warm(3)

            def gate_block():
                # gate: sw = g*sigmoid(g) via exp (same act table), recip on
                # DVE. For blocks 0..NB-2 this is issued mid pass-1 where its
                # tensor work bridges the okp dependency stall; for the last
                # block it runs early so its scalar ops don't delay the rms
                # stats that gate the kernel tail.
                for h in range(2):
                    psg = projb(wg_s, h, nb)
                    e_b = gwork.tile([128, 512], bf16, tag="e")
                    nc.scalar.activation(out=e_b, in_=psg, func=Exp, scale=-1.0)
                    g_b = gwork.tile([128, 512], bf16, tag="g")
                    nc.scalar.copy(out=g_b, in_=psg)
                    t1 = gwork.tile([128, 512], f32, tag="t1")
                    nc.vector.tensor_scalar(out=t1, in0=e_b, scalar1=1.0,
                                            scalar2=None, op0=mybir.AluOpType.add)
                    t2 = gwork.tile([128, 512], f32, tag="t2")
                    nc.vector.reciprocal_approx_fast(out=t2, in_=t1)
                    nc.vector.tensor_mul(sw[:, h, blk], g_b, t2)

            if g == NB - 1:
                gate_block()

            # time-major transposes of w and k for this block's chunks
            for tcb in range(g * GRP, (g + 1) * GRP):
                cblk = slice(tcb * 128, (tcb + 1) * 128)
                pst = ps_tr.tile([128, 128], bf16, tag="tr")
                nc.tensor.transpose(pst, wT2[:, cblk], ident)
                nc.vector.tensor_copy(out=w_tm[:, tcb, :], in_=pst)
                pst = ps_tr.tile([128, 128], bf16, tag="tr")
                nc.tensor.transpose(pst, kT2[:, cblk], ident)
                nc.scalar.copy(out=k_tm[:, tcb, :], in_=pst)

            # snapshot prefix: snaps[c] = state after chunks 0..c
            for tcb in range(g * GRP, min((g + 1) * GRP, NCH - 1)):
                u_ps = ps_mm.tile([128, 256], f32, tag="mm")
                for h in range(2):
                    hp = slice(h * 64, (h + 1) * 64)
                    nc.tensor.matmul(u_ps[hp, 0:64], lhsT=k_tm[:, tcb, hp],
                                     rhs=w_tm[:, tcb, hp], start=True, stop=True)
                for h in range(2):
                    hp = slice(h * 64, (h + 1) * 64)
                    nc.tensor.matmul(u_ps[hp, 64:192],
                                     lhsT=w_tm[:, tcb, hp],
                                     rhs=v_tm[:, tcb, h * 128:(h + 1) * 128],
                                     start=True, stop=True)
                snapf = snapp.tile([128, 192], f32, tag="snapf")
                if tcb == 0:
                    nc.vector.tensor_copy(out=snapf, in_=u_ps[:, 0:192])
                else:
                    nc.vector.tensor_add(snapf, state["snapf_prev"],
                                         u_ps[:, 0:192])
                snapb = snapstore.tile([128, 192], bf16, tag=f"s{tcb}")
                nc.gpsimd.tensor_copy(out=snapb, in_=snapf)
                state["snapf_prev"] = snapf
                snaps.append(snapb)

            # ---- pass 1 for this block's chunks (stage-major) ----
            chunks = range(g * GRP, (g + 1) * GRP)
            blks = {c: slice(c * 128, (c + 1) * 128) for c in chunks}

            # slot logits per head: atm[j, t] = mask * k^T q
            aps_t, atm_t = {}, {}
            for c in chunks:
                for h in range(2):
                    hp = slice(h * 64, (h + 1) * 64)
                    aps = ps_mm.tile([128, 128], f32, tag="mm")
                    nc.tensor.matmul(aps, lhsT=kT2[hp, blks[c]],
                                     rhs=qT2[hp, blks[c]], start=True, stop=True)
                    aps_t[c, h] = aps
            for c in chunks:
                atm = work.tile([128, 256], bf16, tag="atm")
                for h in range(2):
                    nc.vector.tensor_mul(atm[:, h * 128:(h + 1) * 128],
                                         aps_t[c, h], maskT)
                atm_t[c] = atm

            if g < NB - 1:
                gate_block()

            if prev:
                pass2_chunk(prev[2])

            okp_t = {}
            for c in chunks:
                okp = ps_mm.tile([128, 128], f32, tag="mm")
                # issue the two heads' matmuls adjacently per stage so the
                # col-tiled halves can run concurrently in the PE array
                for h in range(2):
                    hp = slice(h * 64, (h + 1) * 64)
                    nc.tensor.matmul(okp[hp, :], lhsT=w_tm[:, c, hp],
                                     rhs=atm_t[c][:, h * 128:(h + 1) * 128],
                                     start=True, stop=c == 0)
                if c > 0:
                    for h in range(2):
                        hp = slice(h * 64, (h + 1) * 64)
                        nc.tensor.matmul(okp[hp, :], lhsT=snaps[c - 1][hp, 0:64],
                                         rhs=qT2[hp, blks[c]],
                                         start=False, stop=True)
                okp_t[c] = okp
            eok_t = {}
            for c in chunks:
                ok_n = work.tile([128, 128], f32, tag="okn")
                nc.vector.tensor_mul(ok_n, okp_t[c], WinvT2[:, blks[c]])
                eok_t[c] = ok_n
            for c in chunks:
                eok = work.tile([128, 128], bf16, tag="eok")
                nc.scalar.activation(out=eok, in_=eok_t[c], func=Exp)
                eok_t[c] = eok

            # deferred softmax denominator: dsq = EPS * den^2 per head
            pde_t, dsq_t, pvw_t = {}, {}, {}
            for c in chunks:
                pde = ps_tr.tile([128, 128], bf16, tag="tr")
                nc.tensor.transpose(pde, eok_t[c], ident)
                pde_t[c] = pde
            for c in chunks:
                dn = scal.tile([128, 2], f32, tag="dn")
                for h in range(2):
                    nc.vector.tensor_reduce(out=dn[:, h:h + 1],
                                            in_=pde_t[c][:, h * 64:(h + 1) * 64],
                                            axis=mybir.AxisListType.X,
                                            op=mybir.AluOpType.add)
                dsq = scal.tile([128, 2], f32, tag="dsq")
                nc.vector.tensor_scalar(out=dsq, in0=dn, scalar1=EPS,
                                        scalar2=None, op0=mybir.AluOpType.mult)
                nc.vector.tensor_mul(dsq, dsq, dn)
                dsq_t[c] = dsq
                pvw = work.tile([128, 128], bf16, tag="pvw")
                nc.vector.tensor_mul(pvw, eok_t[c], WinvT2[:, blks[c]])
                pvw_t[c] = pvw

            if prev:
                pass2_chunk(prev[3])

            pps_t, ptm_t = {}, {}
            for c in chunks:
                for h in range(2):
                    hp = slice(h * 64, (h + 1) * 64)
                    pps = ps_mm.tile([128, 128], f32, tag="mm")
                    nc.tensor.matmul(pps, lhsT=wT2[hp, blks[c]],
                                     rhs=pvw_t[c][hp, :], start=True, stop=True)
                    pps_t[c, h] = pps
            for c in chunks:
                ptm = work.tile([128, 256], bf16, tag="ptm")
                for h in range(2):
                    nc.vector.tensor_mul(ptm[:, h * 128:(h + 1) * 128],
                                         pps_t[c, h], maskT)
                ptm_t[c] = ptm

            # ovp matmuls with the rms-stat chain issued per chunk right
            # behind them, so the scalar queue drains while later chunks'
            # matmuls still run (shortens the kernel tail).
            for c in chunks:
                ovp_t = {}
                for h in range(2):
                    hp = slice(h * 64, (h + 1) * 64)
                    ovp = ps_mm.tile([128, 128], f32, tag="mm")
                    nc.tensor.matmul(ovp,
                                     lhsT=ptm_t[c][:, h * 128:(h + 1) * 128],
                                     rhs=v_tm[:, c, h * 128:(h + 1) * 128],
                                     start=True, stop=c == 0)
                    if c > 0:
                        nc.tensor.matmul(ovp, lhsT=pvw_t[c][hp, :],
                                         rhs=snaps[c - 1][hp, 64:192],
                                         start=False, stop=True)
                    ovp_t[h] = ovp
                msq = scal.tile([128, 2], f32, tag="msq")
                for h in range(2):
                    hb = slice(h * 128, (h + 1) * 128)
                    nc.vector.tensor_copy(out=ov_all[:, c, hb], in_=ovp_t[h])
                    # rms stats: ms = sum(ov^2)/DV + EPS*den^2
                    scr = work.tile([128, 128], bf16, tag="scr")
                    nc.scalar.activation(out=scr, in_=ovp_t[h], func=Square,
                                         accum_out=msq[:, h:h + 1])
                    nc.vector.tensor_scalar(
                        out=ms_all[:, c, h:h + 1], in0=msq[:, h:h + 1],
                        scalar1=1.0 / DV, scalar2=dsq_t[c][:, h:h + 1],
                        op0=mybir.AluOpType.mult, op1=mybir.AluOpType.add)
                # rstd = exp(-0.5 * ln(ms)) -- same act table as Exp; issued
                # here so the last chunks' rstd is ready when pass 2 starts
                lnb = scal.tile([128, 2], f32, tag="lnb")
                nc.scalar.activation(out=lnb, in_=ms_all[:, c, :], func=Ln)
                nc.scalar.activation(out=rstd_all[:, c, :], in_=lnb, func=Exp,
                                     scale=-0.5)

        # ---- tail: pass 2 of the final block. A few junk matmuls bridge the
        # dependency wait on the last chunks' rms stats so the PE stays in
        # the fast pstate for the final out-projections. ----
        for i, c in enumerate(range(NCH - GRP, NCH)):
            pe_warm(12 if i == 0 else 0)
            pass2_chunk(c)

    nc.compile()
    return nc


def _get_nc():
    if "nc" not in _CACHE:
        _CACHE["nc"] = _build()
    return _CACHE["nc"]


def _part_major(a, inner):
    # [D, cols] -> [128, DT, cols] with partition-major contiguous runs
    return np.ascontiguousarray(
        a.reshape(DT, 128, inner).transpose(1, 0, 2))


def _make_in_maps(inputs):
    import ml_dtypes

    bfdt = ml_dtypes.bfloat16
    f8dt = ml_dtypes.float8_e4m3

    hs = np.asarray(inputs["hidden_states"], dtype=np.float32).reshape(T, D)
    # [T, D] -> [128, NB, DT, 512]: element (p, nb, dd, t') = hs[nb*512+t', dd*128+p]
    hs_pm = hs.reshape(NB, 512, DT, 128).transpose(3, 0, 2, 1)
    hst8 = np.ascontiguousarray((hs_pm * HS_S)).astype(f8dt)
    hsb = np.ascontiguousarray(hs_pm).astype(bfdt)

    Wq = np.asarray(inputs["Wq"], dtype=np.float32)
    Wk = np.asarray(inputs["Wk"], dtype=np.float32)
    Wv = np.asarray(inputs["Wv"], dtype=np.float32)
    Wg = np.asarray(inputs["Wg"], dtype=np.float32)
    Ws = np.asarray(inputs["Ws"], dtype=np.float32)
    Wo = np.asarray(inputs["Wo"], dtype=np.float32)
    gnw = np.asarray(inputs["g_norm_weight"], dtype=np.float32)
    # fold gnw into Wo rows: (o_n*gnw*sg) @ Wo == (o_n*sg) @ (gnw[:,None]*Wo)
    Wo_f = Wo * np.tile(gnw, H)[:, None]

    in_maps = []
    for i in range(N_CORES):
        in_maps.append({
            "hst8": hst8,
            "hsb": hsb,
            "wq": _part_major(Wq[:, i * 128:(i + 1) * 128] * W_S, 128).astype(f8dt),
            "wk": _part_major(Wk[:, i * 128:(i + 1) * 128] * W_S, 128).astype(f8dt),
            "ws": _part_major(Ws[:, i * 128:(i + 1) * 128] * W_S, 128).astype(f8dt),
            "wv": _part_major(Wv[:, i * 256:(i + 1) * 256], 256).astype(bfdt),
            "wg": _part_major(Wg[:, i * 256:(i + 1) * 256], 256).astype(bfdt),
            "wo": np.ascontiguousarray(
                Wo_f[i * 256:(i + 1) * 256, :].reshape(2, 128, D)
                .transpose(1, 0, 2)).astype(bfdt),
        })
    return in_maps


def _gather(res):
    out = np.zeros((T, D), np.float32)
    for r in res.results:
        out += np.asarray(r["out"]).astype(np.float32)
    return out.reshape(1, T, D)


def kernel(**inputs):
    from concourse.bass_utils import run_bass_kernel_spmd

    nc = _get_nc()
    in_maps = _make_in_maps(inputs)
    res = run_bass_kernel_spmd(nc, in_maps, core_ids=list(range(N_CORES)))
    return _gather(res)
